# revision 1
# baseline (speedup 1.0000x reference)
"""Trainium2 Bass kernel for a binarized ResNet BasicBlock.

Computes (training-mode BN with batch stats, 8-core data parallel over batch):
    out = hardtanh(BN2(conv3x3(sign(hardtanh(BN1(conv3x3(sign(x), sign(w1))))), sign(w2)) + x))

Key facts exploited:
  - Conv inputs are sign() outputs (+-1, 0 at padding) and weights are sign()
    outputs (+-1): bf16 operands are exact, PSUM accumulation is fp32 =>
    conv results are bit-exact integers.
  - sign(hardtanh(BN1(y))) == sign(gamma1*(y-m1)*rstd1 + beta1); with beta1 == 0
    this is sign(gamma1*(y - m1)) -- only the batch MEAN of conv1's output is
    needed cross-core, not the variance.
  - BN batch stats need global (all 32 images) moments: tiny per-core partial
    sums are AllGather'ed ([128,1] or [128,2] fp32) and summed on-chip.
"""

import os
import sys

sys.path.insert(0, "/opt/trn_rl_repo")

import numpy as np
import ml_dtypes

import concourse.bass as bass
import concourse.bacc as bacc
import concourse.tile as tile
from concourse import mybir
from concourse.bass_utils import run_bass_kernel_spmd

N_CORES = 8
B = 32
NB = B // N_CORES          # images per core
C = 128
H = W = 56
HP = 58                    # padded row pitch of the sign maps
NPIX = H * W               # 3136
SLAB = 3376                # per-image padded sign-map size (58*58=3364 + margin)
RPC = 8                    # output rows per PSUM chunk
CHUNK = RPC * HP           # 464 matmul columns per chunk
NCHUNK = H // RPC          # 7 chunks per image
VALID = RPC * W            # 448 valid columns per chunk
NTOT = B * NPIX            # 100352 elements per channel in the global batch
EPS = 1e-5

F32 = mybir.dt.float32
BF16 = mybir.dt.bfloat16
AF = mybir.ActivationFunctionType
ALU = mybir.AluOpType

# exec time of the last device launch (ns), when profiling is enabled
last_exec_time_ns = None
last_profile = None

_built = {}


def _taps():
    for kh in range(3):
        for kw in range(3):
            yield kh * 3 + kw, kh * HP + kw


def _build(beta1_zero: bool, beta2_zero: bool):
    nc = bacc.Bacc("TRN2", target_bir_lowering=False, debug=False,
                   num_devices=N_CORES)

    x_d = nc.dram_tensor("x", [NB, C, H, W], F32, kind="ExternalInput").ap()
    w1_d = nc.dram_tensor("w1s", [C, 9, C], BF16, kind="ExternalInput").ap()
    w2_d = nc.dram_tensor("w2s", [C, 9, C], BF16, kind="ExternalInput").ap()
    g1_d = nc.dram_tensor("g1", [C, 1], F32, kind="ExternalInput").ap()
    g2_d = nc.dram_tensor("g2", [C, 1], F32, kind="ExternalInput").ap()
    be1_d = be2_d = None
    if not beta1_zero:
        be1_d = nc.dram_tensor("be1", [C, 1], F32, kind="ExternalInput").ap()
    if not beta2_zero:
        be2_d = nc.dram_tensor("be2", [C, 1], F32, kind="ExternalInput").ap()
    out_d = nc.dram_tensor("out", [NB, C, H, W], F32, kind="ExternalOutput").ap()

    nst = 2 if not beta1_zero else 1  # stats per chunk for conv1 (sum[, sumsq])

    with tile.TileContext(nc) as tc:
        with (
            tc.tile_pool(name="big", bufs=1) as big,
            tc.tile_pool(name="small", bufs=1) as small,
            tc.tile_pool(name="scr", bufs=2) as scr,
            tc.tile_pool(name="ps1p", bufs=3, space="PSUM") as ps1p,
            tc.tile_pool(name="ps2p", bufs=3, space="PSUM") as ps2p,
            tc.tile_pool(name="dram", bufs=1, space="DRAM") as dram,
        ):
            xs = big.tile([C, NB, NPIX], F32)      # input (later: pre-clip temp)
            zb = big.tile([C, NB, NPIX], F32)      # y1, then z = y2 + x, then out
            A1 = big.tile([C, NB, SLAB], BF16)     # padded sign(x)
            A2 = big.tile([C, NB, SLAB], BF16)     # padded sign(BN1 out)
            w1t = big.tile([C, 9, C], BF16)
            w2t = big.tile([C, 9, C], BF16)

            S1s = small.tile([C, NB * NCHUNK, nst], F32)
            st2 = small.tile([C, NB * NCHUNK, 6], F32)
            g1sb = small.tile([C, 1], F32)
            g2sb = small.tile([C, 1], F32)
            eps_t = small.tile([C, 1], F32)

            # ---- preamble ----
            nc.vector.memset(A1[:], 0.0)
            nc.vector.memset(A2[:], 0.0)
            nc.vector.memset(eps_t, EPS)
            nc.sync.dma_start(out=w1t, in_=w1_d)
            nc.sync.dma_start(out=w2t, in_=w2_d)
            nc.sync.dma_start(out=g1sb, in_=g1_d)
            nc.sync.dma_start(out=g2sb, in_=g2_d)
            ng1 = small.tile([C, 1], F32)
            nc.vector.tensor_scalar_mul(ng1, g1sb, -1.0)
            if not beta1_zero:
                be1sb = small.tile([C, 1], F32)
                nc.sync.dma_start(out=be1sb, in_=be1_d)
            if not beta2_zero:
                be2sb = small.tile([C, 1], F32)
                nc.sync.dma_start(out=be2sb, in_=be2_d)

            for n in range(NB):
                nc.sync.dma_start(
                    out=xs[:, n, :].rearrange("p (h w) -> p h w", w=W),
                    in_=x_d[n],
                )

            def interior(Abuf, n):
                v = Abuf[:, n, : HP * HP].rearrange("p (h w) -> p h w", w=HP)
                return v[:, 1 : 1 + H, 1 : 1 + W]

            # ---- sign(x) into padded A1 ----
            for n in range(NB):
                nc.scalar.activation(
                    out=interior(A1, n),
                    in_=xs[:, n, :].rearrange("p (h w) -> p h w", w=W),
                    func=AF.Sign,
                )

            # ---- conv1 (9 shifted matmuls / chunk) + drain + partial sums ----
            for n in range(NB):
                for cch in range(NCHUNK):
                    ps_t = ps1p.tile([C, CHUNK], F32, name="ps1")
                    j0 = cch * CHUNK
                    for t, off in _taps():
                        nc.tensor.matmul(
                            ps_t,
                            w1t[:, t, :],
                            A1[:, n, j0 + off : j0 + off + CHUNK],
                            start=(t == 0),
                            stop=(t == 8),
                        )
                    pv = ps_t.rearrange("p (r w) -> p r w", w=HP)[:, :, :W]
                    dst = zb[:, n, cch * VALID : (cch + 1) * VALID].rearrange(
                        "p (r w) -> p r w", w=W
                    )
                    idx = n * NCHUNK + cch
                    nc.scalar.activation(
                        out=dst, in_=pv, func=AF.Copy,
                        accum_out=S1s[:, idx, 0:1],
                    )
                    if not beta1_zero:
                        sqs = scr.tile([C, RPC, W], F32, name="sqscr")
                        nc.scalar.activation(
                            out=sqs, in_=pv, func=AF.Square,
                            accum_out=S1s[:, idx, 1:2],
                        )

            # ---- global mean (and var if needed) of y1 via AllGather ----
            red1 = small.tile([C, nst], F32)
            nc.vector.tensor_reduce(
                out=red1,
                in_=S1s.rearrange("p k s -> p s k"),
                axis=mybir.AxisListType.X,
                op=ALU.add,
            )
            cc1_in = dram.tile([C, nst], F32)
            cc1_out = dram.tile([N_CORES * C, nst], F32)
            nc.sync.dma_start(out=cc1_in, in_=red1)
            nc.gpsimd.collective_compute(
                "AllGather", ALU.bypass,
                replica_groups=[list(range(N_CORES))],
                ins=[cc1_in[:]], outs=[cc1_out[:]],
            )
            g1g = small.tile([C, nst, N_CORES], F32)
            nc.sync.dma_start(
                out=g1g,
                in_=bass.AP(
                    tensor=cc1_out.tensor,
                    offset=cc1_out.offset,
                    ap=[[nst, C], [1, nst], [C * nst, N_CORES]],
                ),
            )
            tot1 = small.tile([C, nst], F32)
            nc.vector.tensor_reduce(
                out=tot1, in_=g1g, axis=mybir.AxisListType.X, op=ALU.add
            )
            m1 = small.tile([C, 1], F32)
            nc.vector.tensor_scalar_mul(m1, tot1[:, 0:1], 1.0 / NTOT)
            b1t = small.tile([C, 1], F32)
            if beta1_zero:
                # sign(g1*(y1-m1)) -> scale=g1, bias=-g1*m1
                s1sc = g1sb
                nc.vector.tensor_tensor(out=b1t, in0=m1, in1=ng1, op=ALU.mult)
            else:
                e1 = small.tile([C, 1], F32)
                nc.vector.tensor_scalar_mul(e1, tot1[:, 1:2], 1.0 / NTOT)
                m1sq = small.tile([C, 1], F32)
                nc.vector.tensor_tensor(out=m1sq, in0=m1, in1=m1, op=ALU.mult)
                v1 = small.tile([C, 1], F32)
                nc.vector.tensor_tensor(out=v1, in0=e1, in1=m1sq, op=ALU.subtract)
                sd1 = small.tile([C, 1], F32)
                nc.scalar.activation(out=sd1, in_=v1, func=AF.Sqrt, bias=eps_t)
                r1 = small.tile([C, 1], F32)
                nc.vector.reciprocal(r1, sd1)
                s1sc = small.tile([C, 1], F32)
                nc.vector.tensor_tensor(out=s1sc, in0=g1sb, in1=r1, op=ALU.mult)
                sm = small.tile([C, 1], F32)
                nc.vector.tensor_tensor(out=sm, in0=s1sc, in1=m1, op=ALU.mult)
                nc.vector.tensor_tensor(out=b1t, in0=be1sb, in1=sm, op=ALU.subtract)

            # ---- sign(BN1(y1)) into padded A2 ----
            for n in range(NB):
                nc.scalar.activation(
                    out=interior(A2, n),
                    in_=zb[:, n, :].rearrange("p (h w) -> p h w", w=W),
                    func=AF.Sign,
                    bias=b1t,
                    scale=s1sc,
                )

            # ---- conv2 + residual add + BN2 partial stats ----
            for n in range(NB):
                for cch in range(NCHUNK):
                    ps_t = ps2p.tile([C, CHUNK], F32, name="ps2")
                    j0 = cch * CHUNK
                    for t, off in _taps():
                        nc.tensor.matmul(
                            ps_t,
                            w2t[:, t, :],
                            A2[:, n, j0 + off : j0 + off + CHUNK],
                            start=(t == 0),
                            stop=(t == 8),
                        )
                    pv = ps_t.rearrange("p (r w) -> p r w", w=HP)[:, :, :W]
                    zv = zb[:, n, cch * VALID : (cch + 1) * VALID]
                    nc.vector.tensor_tensor(
                        out=zv.rearrange("p (r w) -> p r w", w=W),
                        in0=pv,
                        in1=xs[:, n, cch * VALID : (cch + 1) * VALID].rearrange(
                            "p (r w) -> p r w", w=W
                        ),
                        op=ALU.add,
                    )
                    nc.vector.bn_stats(out=st2[:, n * NCHUNK + cch, :], in_=zv)

            # ---- global BN2 stats via AllGather of (sum, sumsq) ----
            mv2 = small.tile([C, 2], F32)
            nc.vector.bn_aggr(out=mv2, in_=st2)
            msq = small.tile([C, 1], F32)
            nc.vector.tensor_tensor(out=msq, in0=mv2[:, 0:1], in1=mv2[:, 0:1],
                                    op=ALU.mult)
            vps = small.tile([C, 1], F32)
            nc.vector.tensor_tensor(out=vps, in0=mv2[:, 1:2], in1=msq, op=ALU.add)
            loc2 = small.tile([C, 2], F32)
            nc.vector.tensor_scalar_mul(loc2[:, 0:1], mv2[:, 0:1], float(NB * NPIX))
            nc.vector.tensor_scalar_mul(loc2[:, 1:2], vps, float(NB * NPIX))
            cc2_in = dram.tile([C, 2], F32)
            cc2_out = dram.tile([N_CORES * C, 2], F32)
            nc.sync.dma_start(out=cc2_in, in_=loc2)
            nc.gpsimd.collective_compute(
                "AllGather", ALU.bypass,
                replica_groups=[list(range(N_CORES))],
                ins=[cc2_in[:]], outs=[cc2_out[:]],
            )
            g2g = small.tile([C, 2, N_CORES], F32)
            nc.sync.dma_start(
                out=g2g,
                in_=bass.AP(
                    tensor=cc2_out.tensor,
                    offset=cc2_out.offset,
                    ap=[[2, C], [1, 2], [C * 2, N_CORES]],
                ),
            )
            tot2 = small.tile([C, 2], F32)
            nc.vector.tensor_reduce(
                out=tot2, in_=g2g, axis=mybir.AxisListType.X, op=ALU.add
            )
            m2 = small.tile([C, 1], F32)
            nc.vector.tensor_scalar_mul(m2, tot2[:, 0:1], 1.0 / NTOT)
            e2 = small.tile([C, 1], F32)
            nc.vector.tensor_scalar_mul(e2, tot2[:, 1:2], 1.0 / NTOT)
            m2sq = small.tile([C, 1], F32)
            nc.vector.tensor_tensor(out=m2sq, in0=m2, in1=m2, op=ALU.mult)
            v2 = small.tile([C, 1], F32)
            nc.vector.tensor_tensor(out=v2, in0=e2, in1=m2sq, op=ALU.subtract)
            sd2 = small.tile([C, 1], F32)
            nc.scalar.activation(out=sd2, in_=v2, func=AF.Sqrt, bias=eps_t)
            r2 = small.tile([C, 1], F32)
            nc.vector.reciprocal(r2, sd2)
            s2 = small.tile([C, 1], F32)
            nc.vector.tensor_tensor(out=s2, in0=g2sb, in1=r2, op=ALU.mult)
            nm2 = small.tile([C, 1], F32)
            if beta2_zero:
                nc.vector.tensor_scalar_mul(nm2, m2, -1.0)
            else:
                # (z + (be2/s2 - m2)) * s2 == (z - m2)*s2 + be2
                rs2 = small.tile([C, 1], F32)
                nc.vector.reciprocal(rs2, s2)
                bos = small.tile([C, 1], F32)
                nc.vector.tensor_tensor(out=bos, in0=be2sb, in1=rs2, op=ALU.mult)
                nc.vector.tensor_tensor(out=nm2, in0=bos, in1=m2, op=ALU.subtract)

            # ---- final affine + hardtanh + store ----
            for n in range(NB):
                u = xs[:, n, :]
                nc.vector.tensor_scalar(
                    u, zb[:, n, :], nm2, s2, ALU.add, ALU.mult
                )
                nc.vector.tensor_scalar(
                    zb[:, n, :], u, -1.0, 1.0, ALU.max, ALU.min
                )
                nc.sync.dma_start(
                    out=out_d[n],
                    in_=zb[:, n, :].rearrange("p (h w) -> p h w", w=W),
                )

    nc.compile()
    return nc


def _get_nc(beta1_zero, beta2_zero):
    key = (beta1_zero, beta2_zero)
    if key not in _built:
        _built[key] = _build(*key)
    return _built[key]


def kernel(x, w1, w2, gamma1, beta1, gamma2, beta2):
    global last_exec_time_ns, last_profile

    x = np.ascontiguousarray(np.asarray(x, dtype=np.float32))
    w1 = np.asarray(w1, dtype=np.float32)
    w2 = np.asarray(w2, dtype=np.float32)
    gamma1 = np.asarray(gamma1, dtype=np.float32).reshape(C, 1)
    gamma2 = np.asarray(gamma2, dtype=np.float32).reshape(C, 1)
    beta1 = np.asarray(beta1, dtype=np.float32).reshape(C, 1)
    beta2 = np.asarray(beta2, dtype=np.float32).reshape(C, 1)

    beta1_zero = bool(np.all(beta1 == 0.0))
    beta2_zero = bool(np.all(beta2 == 0.0))
    nc = _get_nc(beta1_zero, beta2_zero)

    # sign(w)[o,i,kh,kw] transposed to [cin, tap, cout] (exact in bf16)
    def pack_w(w):
        ws = np.sign(w).astype(np.float32)
        ws = ws.transpose(1, 2, 3, 0).reshape(C, 9, C)
        return np.ascontiguousarray(ws.astype(ml_dtypes.bfloat16))

    w1p = pack_w(w1)
    w2p = pack_w(w2)

    in_maps = []
    for c in range(N_CORES):
        m = {
            "x": x[c * NB : (c + 1) * NB],
            "w1s": w1p,
            "w2s": w2p,
            "g1": gamma1,
            "g2": gamma2,
        }
        if not beta1_zero:
            m["be1"] = beta1
        if not beta2_zero:
            m["be2"] = beta2
        in_maps.append(m)

    trace = bool(int(os.environ.get("BASS_KERNEL_PROFILE", "0")))
    res = run_bass_kernel_spmd(nc, in_maps, core_ids=list(range(N_CORES)),
                               trace=trace)
    last_exec_time_ns = res.exec_time_ns
    last_profile = res.profile_json
    out = np.concatenate([res.results[c]["out"] for c in range(N_CORES)], axis=0)
    return out


# revision 3
# speedup vs baseline: 1.2836x; 1.2836x over previous
"""Trainium2 Bass kernel for a binarized ResNet BasicBlock.

Computes (training-mode BN with batch stats, 8-core data parallel over batch):
    out = hardtanh(BN2(conv3x3(sign(hardtanh(BN1(conv3x3(sign(x), sign(w1))))), sign(w2)) + x))

Key facts exploited:
  - Conv inputs are sign() outputs (+-1, 0 at padding) and weights are sign()
    outputs (+-1): bf16/fp8 operands are exact, PSUM accumulation is fp32 =>
    conv results are bit-exact integers.
  - fp8 DoubleRow matmuls process two 3x3 taps per instruction: the rhs is an
    overlapping strided view [C, 2, N] of the padded sign map where the "2"
    dim steps between the two taps' shifted windows.
  - sign(hardtanh(BN1(y))) == sign(gamma1*(y-m1)*rstd1 + beta1); with beta1 == 0
    this is sign(gamma1*(y - m1)) -- only the batch MEAN of conv1's output is
    needed cross-core, not the variance.
  - BN batch stats need global (all 32 images) moments: tiny per-core partial
    sums are AllGather'ed ([128,nst] fp32) and summed on-chip.
"""

import os
import sys

sys.path.insert(0, "/opt/trn_rl_repo")

import numpy as np
import ml_dtypes

import concourse.bass as bass
import concourse.bacc as bacc
import concourse.tile as tile
from concourse import mybir
from concourse.bass_utils import run_bass_kernel_spmd

N_CORES = 8
B = 32
NB = B // N_CORES          # images per core
C = 128
H = W = 56
HP = 58                    # padded row pitch of the sign maps
NPIX = H * W               # 3136
SLAB = 3376                # per-image padded sign-map size (58*58=3364 + margin)
RPC = 8                    # output rows per PSUM chunk
CHUNK = RPC * HP           # 464 matmul columns per chunk
NCHUNK = H // RPC          # 7 chunks per image
VALID = RPC * W            # 448 valid columns per chunk
NTOT = B * NPIX            # 100352 elements per channel in the global batch
EPS = 1e-5
OFFS = [kh * HP + kw for kh in range(3) for kw in range(3)]
GROUPS = [[0, 1, 2], [3, 4, 5], [6]]   # chunk groups sharing one weight pass

F32 = mybir.dt.float32
BF16 = mybir.dt.bfloat16
FP8 = mybir.dt.float8e4
AF = mybir.ActivationFunctionType
ALU = mybir.AluOpType

# "fp8dr": fp8 + DoubleRow (2 taps/matmul, 5 passes); "bf16": 9 single-tap passes
MM_MODE = os.environ.get("BASS_KERNEL_MM", "fp8dr")

# exec time of the last device launch (ns), when profiling is enabled
last_exec_time_ns = None
last_profile = None

_built = {}


def _build(beta1_zero: bool, beta2_zero: bool, mm_mode: str):
    use_dr = mm_mode == "fp8dr"
    adt = FP8 if use_dr else BF16
    if use_dr:
        passes = [(0, 1), (2, 3), (4, 5), (6, 7), (8,)]
    else:
        passes = [(t,) for t in range(9)]

    nc = bacc.Bacc("TRN2", target_bir_lowering=False, debug=False,
                   num_devices=N_CORES)

    x_d = nc.dram_tensor("x", [NB, C, H, W], F32, kind="ExternalInput").ap()
    w1_d = nc.dram_tensor("w1s", [C, 9, C], adt, kind="ExternalInput").ap()
    w2_d = nc.dram_tensor("w2s", [C, 9, C], adt, kind="ExternalInput").ap()
    g1_d = nc.dram_tensor("g1", [C, 1], F32, kind="ExternalInput").ap()
    g2_d = nc.dram_tensor("g2", [C, 1], F32, kind="ExternalInput").ap()
    be1_d = be2_d = None
    if not beta1_zero:
        be1_d = nc.dram_tensor("be1", [C, 1], F32, kind="ExternalInput").ap()
    if not beta2_zero:
        be2_d = nc.dram_tensor("be2", [C, 1], F32, kind="ExternalInput").ap()
    out_d = nc.dram_tensor("out", [NB, C, H, W], F32, kind="ExternalOutput").ap()

    nst = 2 if not beta1_zero else 1  # stats per chunk for conv1 (sum[, sumsq])

    with tile.TileContext(nc) as tc:
        with (
            tc.tile_pool(name="big", bufs=1) as big,
            tc.tile_pool(name="small", bufs=1) as small,
            tc.tile_pool(name="scr", bufs=2) as scr,
            tc.tile_pool(name="psp", bufs=6, space="PSUM") as psp,
            tc.tile_pool(name="dram", bufs=1, space="DRAM") as dram,
        ):
            xs = big.tile([C, NB, NPIX], F32)      # input (later: pre-clip temp)
            zb = big.tile([C, NB, NPIX], F32)      # y1, then z = y2 + x, then out
            A1 = big.tile([C, NB, SLAB], adt)      # padded sign(x)
            A2 = big.tile([C, NB, SLAB], adt)      # padded sign(BN1 out)
            w1t = big.tile([C, 9, C], adt)
            w2t = big.tile([C, 9, C], adt)

            S1s = small.tile([C, NB * NCHUNK, nst], F32)
            st2 = small.tile([C, NB * NCHUNK, 6], F32)
            g1sb = small.tile([C, 1], F32)
            g2sb = small.tile([C, 1], F32)
            eps_t = small.tile([C, 1], F32)

            # ---- preamble ----
            # zero only the halo of the padded sign maps (interior is always
            # overwritten by Sign before any conv reads it)
            for Ab in (A1, A2):
                base = Ab[:, 0, 0:1]
                pap = base.ap[0]
                mk = lambda off, dims: bass.AP(
                    tensor=base.tensor, offset=base.offset + off, ap=[pap] + dims
                )
                # top padded row (+ left col of row 1 comes from the strided set)
                nc.vector.memset(mk(0, [[SLAB, NB], [1, HP]]), 0.0)
                # cols 57,58*r: right col of row r, left col of row r+1
                nc.vector.memset(mk(HP - 1, [[SLAB, NB], [HP, H], [1, 2]]), 0.0)
                # bottom padded row + tail margin
                nc.vector.memset(
                    mk(H * HP + HP - 1, [[SLAB, NB], [1, SLAB - (H * HP + HP - 1)]]),
                    0.0,
                )
            nc.vector.memset(eps_t, EPS)
            nc.sync.dma_start(out=w1t, in_=w1_d)
            nc.sync.dma_start(out=w2t, in_=w2_d)
            nc.sync.dma_start(out=g1sb, in_=g1_d)
            nc.sync.dma_start(out=g2sb, in_=g2_d)
            ng1 = small.tile([C, 1], F32)
            nc.vector.tensor_scalar_mul(ng1, g1sb, -1.0)
            if not beta1_zero:
                be1sb = small.tile([C, 1], F32)
                nc.sync.dma_start(out=be1sb, in_=be1_d)
            if not beta2_zero:
                be2sb = small.tile([C, 1], F32)
                nc.sync.dma_start(out=be2sb, in_=be2_d)

            for n in range(NB):
                nc.sync.dma_start(
                    out=xs[:, n, :].rearrange("p (h w) -> p h w", w=W),
                    in_=x_d[n],
                )

            def interior(Abuf, n):
                v = Abuf[:, n, : HP * HP].rearrange("p (h w) -> p h w", w=HP)
                return v[:, 1 : 1 + H, 1 : 1 + W]

            def conv(Abuf, wt, n, grp, drain_fn):
                """9-tap binary conv for chunk group `grp` of image n.
                Tap-outer so consecutive matmuls share the stationary weights."""
                pts = [psp.tile([C, CHUNK], F32, name="psc") for _ in grp]
                for pi, tp in enumerate(passes):
                    kw = dict(start=(pi == 0), stop=(pi == len(passes) - 1))
                    for pt, cch in zip(pts, grp):
                        j0 = cch * CHUNK
                        if len(tp) == 2:
                            oa, ob = OFFS[tp[0]], OFFS[tp[1]]
                            bse = Abuf[:, n, j0 + oa : j0 + oa + CHUNK]
                            rhs = bass.AP(
                                tensor=bse.tensor,
                                offset=bse.offset,
                                ap=[bse.ap[0], [ob - oa, 2], [1, CHUNK]],
                            )
                            nc.tensor.matmul(
                                pt, wt[:, tp[0] : tp[0] + 2, :], rhs,
                                perf_mode=mybir.MatmulPerfMode.DoubleRow, **kw,
                            )
                        else:
                            t = tp[0]
                            nc.tensor.matmul(
                                pt, wt[:, t, :],
                                Abuf[:, n, j0 + OFFS[t] : j0 + OFFS[t] + CHUNK],
                                **kw,
                            )
                for pt, cch in zip(pts, grp):
                    drain_fn(n, cch, pt)

            # ---- sign(x) into padded A1 ----
            for n in range(NB):
                nc.scalar.activation(
                    out=interior(A1, n),
                    in_=xs[:, n, :].rearrange("p (h w) -> p h w", w=W),
                    func=AF.Sign,
                )

            # ---- conv1 + drain (ACT copy with fused per-channel sums) ----
            def drain1(n, cch, pt):
                pv = pt.rearrange("p (r w) -> p r w", w=HP)[:, :, :W]
                dst = zb[:, n, cch * VALID : (cch + 1) * VALID].rearrange(
                    "p (r w) -> p r w", w=W
                )
                idx = n * NCHUNK + cch
                nc.scalar.activation(
                    out=dst, in_=pv, func=AF.Copy, accum_out=S1s[:, idx, 0:1]
                )
                if not beta1_zero:
                    sqs = scr.tile([C, RPC, W], F32, name="sqscr")
                    nc.scalar.activation(
                        out=sqs, in_=pv, func=AF.Square, accum_out=S1s[:, idx, 1:2]
                    )

            for n in range(NB):
                for grp in GROUPS:
                    conv(A1, w1t, n, grp, drain1)

            # ---- global mean (and var if needed) of y1 via AllGather ----
            red1 = small.tile([C, nst], F32)
            nc.vector.tensor_reduce(
                out=red1,
                in_=S1s.rearrange("p k s -> p s k"),
                axis=mybir.AxisListType.X,
                op=ALU.add,
            )
            cc1_in = dram.tile([C, nst], F32)
            cc1_out = dram.tile([N_CORES * C, nst], F32)
            nc.sync.dma_start(out=cc1_in, in_=red1)
            nc.gpsimd.collective_compute(
                "AllGather", ALU.bypass,
                replica_groups=[list(range(N_CORES))],
                ins=[cc1_in[:]], outs=[cc1_out[:]],
            )
            g1g = small.tile([C, nst, N_CORES], F32)
            nc.sync.dma_start(
                out=g1g,
                in_=bass.AP(
                    tensor=cc1_out.tensor,
                    offset=cc1_out.offset,
                    ap=[[nst, C], [1, nst], [C * nst, N_CORES]],
                ),
            )
            tot1 = small.tile([C, nst], F32)
            nc.vector.tensor_reduce(
                out=tot1, in_=g1g, axis=mybir.AxisListType.X, op=ALU.add
            )
            m1 = small.tile([C, 1], F32)
            nc.vector.tensor_scalar_mul(m1, tot1[:, 0:1], 1.0 / NTOT)
            b1t = small.tile([C, 1], F32)
            if beta1_zero:
                # sign(g1*(y1-m1)) -> scale=g1, bias=-g1*m1
                s1sc = g1sb
                nc.vector.tensor_tensor(out=b1t, in0=m1, in1=ng1, op=ALU.mult)
            else:
                e1 = small.tile([C, 1], F32)
                nc.vector.tensor_scalar_mul(e1, tot1[:, 1:2], 1.0 / NTOT)
                m1sq = small.tile([C, 1], F32)
                nc.vector.tensor_tensor(out=m1sq, in0=m1, in1=m1, op=ALU.mult)
                v1 = small.tile([C, 1], F32)
                nc.vector.tensor_tensor(out=v1, in0=e1, in1=m1sq, op=ALU.subtract)
                sd1 = small.tile([C, 1], F32)
                nc.scalar.activation(out=sd1, in_=v1, func=AF.Sqrt, bias=eps_t)
                r1 = small.tile([C, 1], F32)
                nc.vector.reciprocal(r1, sd1)
                s1sc = small.tile([C, 1], F32)
                nc.vector.tensor_tensor(out=s1sc, in0=g1sb, in1=r1, op=ALU.mult)
                sm = small.tile([C, 1], F32)
                nc.vector.tensor_tensor(out=sm, in0=s1sc, in1=m1, op=ALU.mult)
                nc.vector.tensor_tensor(out=b1t, in0=be1sb, in1=sm, op=ALU.subtract)

            # ---- sign(BN1(y1)) into padded A2 ----
            for n in range(NB):
                nc.scalar.activation(
                    out=interior(A2, n),
                    in_=zb[:, n, :].rearrange("p (h w) -> p h w", w=W),
                    func=AF.Sign,
                    bias=b1t,
                    scale=s1sc,
                )

            # ---- conv2 + residual add + BN2 partial stats ----
            def drain2(n, cch, pt):
                pv = pt.rearrange("p (r w) -> p r w", w=HP)[:, :, :W]
                zv = zb[:, n, cch * VALID : (cch + 1) * VALID]
                nc.vector.tensor_tensor(
                    out=zv.rearrange("p (r w) -> p r w", w=W),
                    in0=pv,
                    in1=xs[:, n, cch * VALID : (cch + 1) * VALID].rearrange(
                        "p (r w) -> p r w", w=W
                    ),
                    op=ALU.add,
                )
                nc.vector.bn_stats(out=st2[:, n * NCHUNK + cch, :], in_=zv)

            for n in range(NB):
                for grp in GROUPS:
                    conv(A2, w2t, n, grp, drain2)

            # ---- global BN2 stats via AllGather of (sum, sumsq) ----
            mv2 = small.tile([C, 2], F32)
            nc.vector.bn_aggr(out=mv2, in_=st2)
            msq = small.tile([C, 1], F32)
            nc.vector.tensor_tensor(out=msq, in0=mv2[:, 0:1], in1=mv2[:, 0:1],
                                    op=ALU.mult)
            vps = small.tile([C, 1], F32)
            nc.vector.tensor_tensor(out=vps, in0=mv2[:, 1:2], in1=msq, op=ALU.add)
            loc2 = small.tile([C, 2], F32)
            nc.vector.tensor_scalar_mul(loc2[:, 0:1], mv2[:, 0:1], float(NB * NPIX))
            nc.vector.tensor_scalar_mul(loc2[:, 1:2], vps, float(NB * NPIX))
            cc2_in = dram.tile([C, 2], F32)
            cc2_out = dram.tile([N_CORES * C, 2], F32)
            nc.sync.dma_start(out=cc2_in, in_=loc2)
            nc.gpsimd.collective_compute(
                "AllGather", ALU.bypass,
                replica_groups=[list(range(N_CORES))],
                ins=[cc2_in[:]], outs=[cc2_out[:]],
            )
            g2g = small.tile([C, 2, N_CORES], F32)
            nc.sync.dma_start(
                out=g2g,
                in_=bass.AP(
                    tensor=cc2_out.tensor,
                    offset=cc2_out.offset,
                    ap=[[2, C], [1, 2], [C * 2, N_CORES]],
                ),
            )
            tot2 = small.tile([C, 2], F32)
            nc.vector.tensor_reduce(
                out=tot2, in_=g2g, axis=mybir.AxisListType.X, op=ALU.add
            )
            m2 = small.tile([C, 1], F32)
            nc.vector.tensor_scalar_mul(m2, tot2[:, 0:1], 1.0 / NTOT)
            e2 = small.tile([C, 1], F32)
            nc.vector.tensor_scalar_mul(e2, tot2[:, 1:2], 1.0 / NTOT)
            m2sq = small.tile([C, 1], F32)
            nc.vector.tensor_tensor(out=m2sq, in0=m2, in1=m2, op=ALU.mult)
            v2 = small.tile([C, 1], F32)
            nc.vector.tensor_tensor(out=v2, in0=e2, in1=m2sq, op=ALU.subtract)
            sd2 = small.tile([C, 1], F32)
            nc.scalar.activation(out=sd2, in_=v2, func=AF.Sqrt, bias=eps_t)
            r2 = small.tile([C, 1], F32)
            nc.vector.reciprocal(r2, sd2)
            s2 = small.tile([C, 1], F32)
            nc.vector.tensor_tensor(out=s2, in0=g2sb, in1=r2, op=ALU.mult)
            # ACT computes z*s2 + bf2 where bf2 = beta2 - m2*s2
            ms = small.tile([C, 1], F32)
            nc.vector.tensor_tensor(out=ms, in0=m2, in1=s2, op=ALU.mult)
            bf2 = small.tile([C, 1], F32)
            if beta2_zero:
                nc.vector.tensor_scalar_mul(bf2, ms, -1.0)
            else:
                nc.vector.tensor_tensor(out=bf2, in0=be2sb, in1=ms, op=ALU.subtract)

            # ---- final affine (ACT) + hardtanh (DVE) + store ----
            for n in range(NB):
                u = xs[:, n, :]
                nc.scalar.activation(
                    out=u, in_=zb[:, n, :], func=AF.Identity, bias=bf2, scale=s2
                )
                nc.vector.tensor_scalar(
                    zb[:, n, :], u, -1.0, 1.0, ALU.max, ALU.min
                )
                nc.sync.dma_start(
                    out=out_d[n],
                    in_=zb[:, n, :].rearrange("p (h w) -> p h w", w=W),
                )

    nc.compile()
    return nc


def _get_nc(beta1_zero, beta2_zero, mm_mode):
    key = (beta1_zero, beta2_zero, mm_mode)
    if key not in _built:
        _built[key] = _build(*key)
    return _built[key]


def kernel(x, w1, w2, gamma1, beta1, gamma2, beta2):
    global last_exec_time_ns, last_profile

    x = np.ascontiguousarray(np.asarray(x, dtype=np.float32))
    w1 = np.asarray(w1, dtype=np.float32)
    w2 = np.asarray(w2, dtype=np.float32)
    gamma1 = np.asarray(gamma1, dtype=np.float32).reshape(C, 1)
    gamma2 = np.asarray(gamma2, dtype=np.float32).reshape(C, 1)
    beta1 = np.asarray(beta1, dtype=np.float32).reshape(C, 1)
    beta2 = np.asarray(beta2, dtype=np.float32).reshape(C, 1)

    beta1_zero = bool(np.all(beta1 == 0.0))
    beta2_zero = bool(np.all(beta2 == 0.0))
    nc = _get_nc(beta1_zero, beta2_zero, MM_MODE)

    # sign(w)[o,i,kh,kw] transposed to [cin, tap, cout] (exact in bf16/fp8)
    np_adt = mybir.dt.np(FP8 if MM_MODE == "fp8dr" else BF16)

    def pack_w(w):
        ws = np.sign(w).astype(np.float32)
        ws = ws.transpose(1, 2, 3, 0).reshape(C, 9, C)
        return np.ascontiguousarray(ws.astype(np_adt))

    w1p = pack_w(w1)
    w2p = pack_w(w2)

    in_maps = []
    for c in range(N_CORES):
        m = {
            "x": x[c * NB : (c + 1) * NB],
            "w1s": w1p,
            "w2s": w2p,
            "g1": gamma1,
            "g2": gamma2,
        }
        if not beta1_zero:
            m["be1"] = beta1
        if not beta2_zero:
            m["be2"] = beta2
        in_maps.append(m)

    trace = bool(int(os.environ.get("BASS_KERNEL_PROFILE", "0")))
    res = run_bass_kernel_spmd(nc, in_maps, core_ids=list(range(N_CORES)),
                               trace=trace)
    last_exec_time_ns = res.exec_time_ns
    last_profile = res.profile_json
    out = np.concatenate([res.results[c]["out"] for c in range(N_CORES)], axis=0)
    return out
